# revision 32
# baseline (speedup 1.0000x reference)
"""Trainium2 Bass kernel: topk-masked pseudo-diagonal linear layer.

Math:  a = dykstra_topk(alpha);  W[r,c] = a[(r-c)%n] * V[(r-c)%n, c];
       out = x @ W.T,   with n = 8192, x [1024, 8192], V [8192, 8192].

Strategy (8 NeuronCores, SPMD, no collectives):
  - 2D shard: 4-way over out-features r (R=2048/core) x 2-way over batch
    (BB=512/core).  Each core computes out[b0:b0+512, r0:r0+2048].
  - Host does the cheap, layout-bound work: Dykstra projection of alpha
    (0.4 MFLOP) and the band gather B[c, j] = a[d] * V[d, c] with
    d = (r0 + j - c) % n, emitted in bf16.  The device is a pure
    streaming matmul at the bf16 PE rate: out_tile = xT_block^T @
    B_tile, accumulated over 64 c-tiles in 8 PSUM banks with pipelined
    bf16 LDWEIGHTS.
  - DMA: one queue sustains only ~75 GB/s, and the h0 phase needs
    ~218 GB/s (B stream + resident-x fill), so bulk loads round-robin
    over 5 queues: sync/scalar HWDGE + SWDGE q0 (dma_start) + SWDGE
    q1/q2 (direct dma_gather of 2KB/1KB rows via an iota row-index
    table).  x loads are sliced into 128-256 KB pieces so B tiles
    behind them in a queue are never delayed much.
"""

import math
import numpy as np

# ---- problem constants (hardcoded; must match reference.py) ----
N = 8192
BATCH = 1024
KTOP = math.ceil((1.0 - 0.9) * N * N / N)  # 820
LR = 0.05
ITERS = 50

CFG_FULL = dict(N=N, BB=512, R=2048, TN=512, GATHER_LANES=0)


def dykstra_host(alpha):
    """Euclidean projection of alpha/LR onto {p: 0<=p<=1, sum p = K} via
    the same 50 Dykstra iterations as the reference (f64 accumulate)."""
    x0 = alpha.astype(np.float64) / LR
    n = x0.shape[0]
    v = x0.copy()
    p = np.zeros_like(v)
    q = np.zeros_like(v)
    for _ in range(ITERS):
        t = v + p
        y = t + (KTOP - t.sum()) / n
        p = t - y
        yq = y + q
        v = np.clip(yq, 0.0, 1.0)
        q = yq - v
    return v.astype(np.float32)


def build_nc(cfg=CFG_FULL):
    """Build + compile the single-core SPMD Bass program."""
    import concourse.bass as bass
    import concourse.tile as tile
    from concourse import bacc, mybir

    f32 = mybir.dt.float32
    bf16 = mybir.dt.bfloat16
    i16 = mybir.dt.int16
    Alu = mybir.AluOpType

    n, bb, r_sh, tn = cfg["N"], cfg["BB"], cfg["R"], cfg["TN"]
    glanes = cfg["GATHER_LANES"]
    rhalf = r_sh // 2
    nct = n // 128          # c-tiles
    nbt = bb // 128         # b-tiles
    nrt = rhalf // tn       # r-subtiles per half
    assert nbt * nrt <= 8
    # x pieces, all on the dedicated scalar ring: small first pieces so the
    # first matmuls start early, 1 MB steady pieces after
    xpieces = (2, 2, 4, 8, 8, 8, 8, 8, 8, 8)
    assert sum(xpieces) == nct

    nc = bacc.Bacc(
        "TRN2", target_bir_lowering=False, debug=False, enable_asserts=False,
        num_swdge_queues=1 + glanes,
    )
    xtf_in = nc.dram_tensor("xtf_in", [n, bb], bf16, kind="ExternalInput").ap()
    bt_in = nc.dram_tensor("bt_in", [n, r_sh], bf16, kind="ExternalInput").ap()
    out_d = nc.dram_tensor("out_d", [bb, r_sh], bf16, kind="ExternalOutput").ap()

    with tile.TileContext(nc) as tc:
        with (
            tc.tile_pool(name="idx", bufs=1) as idxp,
            tc.tile_pool(name="xt", bufs=1) as xtp,
            tc.tile_pool(name="vt", bufs=10) as vtp,
            tc.tile_pool(name="vt0", bufs=1) as vtp0,
            tc.tile_pool(name="ps", bufs=8, space=bass.MemorySpace.PSUM) as psp,
            tc.tile_pool(name="st", bufs=4) as stp,
        ):
            # row-index table for dma_gather lanes: idx[p, c] = 16*c + p
            # (row g of a gather lands at table [p=g%16, col=g//16]); clamp
            # to n-1 so the unused partitions p>=16 stay in bounds
            idx_tab = None
            if glanes:
                idx_tab = idxp.tile([128, n // 16], i16)
                nc.gpsimd.iota(
                    idx_tab[:], pattern=[[16, n // 16]], base=0,
                    channel_multiplier=1,
                )
                nc.gpsimd.tensor_scalar(
                    idx_tab[:], idx_tab[:], float(n - 1), None, op0=Alu.min
                )

            # 5 bulk-load lanes, each ~75 GB/s
            def load(dst, src_rows_dram, row0, nrows, elem, estep, lane):
                """dst[128, nrows/128, elem] <- rows [row0, row0+nrows) of a
                DRAM view whose rows are `elem` bf16 wide with pitch `estep`."""
                if lane <= 2:
                    eng = (nc.sync, nc.scalar, nc.gpsimd)[lane]
                    eng.dma_start(
                        dst, src_rows_dram[row0 : row0 + nrows, :].rearrange(
                            "(ct p) b -> p ct b", p=128
                        )
                    )
                else:
                    nc.gpsimd.dma_gather(
                        dst,
                        src_rows_dram,
                        idx_tab[:, row0 // 16 : row0 // 16 + nrows // 16],
                        num_idxs=nrows,
                        num_idxs_reg=nrows,
                        elem_size=elem,
                        elem_step=estep,
                        queue_num=lane - 2,
                    )

            # resident xT piece tiles
            xt_tiles = []
            xt_of_ct = {}
            ct0 = 0
            for xc, cpc in enumerate(xpieces):
                xt_sb = xtp.tile([128, cpc, bb], bf16, name=f"xt{xc}")
                xt_tiles.append((xt_sb, ct0, cpc))
                for ci in range(cpc):
                    xt_of_ct[ct0 + ci] = (xt_sb, ci)
                ct0 += cpc

            def load_xt(xc):
                xt_sb, c0, cpc = xt_tiles[xc]
                load(xt_sb[:], xtf_in, 128 * c0, 128 * cpc, bb, bb, 1)

            bt_half = [
                bt_in[:, rhalf * h : rhalf * (h + 1)] for h in range(2)
            ]

            # one B tile per c-tile (2-ct segments measured slower)
            vt_segs = {
                0: [(ct, 1) for ct in range(nct)],
                1: [(ct, 1) for ct in range(nct)],
            }
            vt_tiles = {}

            def load_vt(h, si):
                c0, cn = vt_segs[h][si]
                # h0: vt alternates the sync + gpsimd rings (scalar is
                # saturated by the x fill); h1: scalar is free, use 3 rings
                lane = (0, 2)[si % 2] if h == 0 else (0, 2, 1)[si % 3]
                vt_t = vtp.tile([128, cn, rhalf], bf16, tag="vt")
                vt_tiles[(h, si)] = vt_t
                load(vt_t[:], bt_half[h], 128 * c0, 128 * cn, rhalf, r_sh, lane)

            # ---- prefetch ramp ----
            # first B tile split across both rings by rt-half so the very
            # first matmul only waits on a 128 KB transfer
            vt00 = vtp0.tile([128, 1, rhalf], bf16)
            nc.sync.dma_start(
                vt00[:, 0, 0:tn],
                bt_half[0][0:128, 0:tn].rearrange("(ct p) b -> p ct b", p=128),
            )
            nc.gpsimd.dma_start(
                vt00[:, 0, tn : 2 * tn],
                bt_half[0][0:128, tn : 2 * tn].rearrange(
                    "(ct p) b -> p ct b", p=128
                ),
            )
            vt_tiles[(0, 0)] = vt00
            load_vt(0, 1)
            for xc in range(len(xpieces)):
                load_xt(xc)

            # ---- main loop ----
            for h in range(2):
                ps_tiles = [
                    psp.tile([128, tn], f32, tag="mm", name=f"ps_{h}_{i}")
                    for i in range(nbt * nrt)
                ]
                for si, (c0, cn) in enumerate(vt_segs[h]):
                    if (h, si) not in vt_tiles:
                        load_vt(h, si)
                    vt_t = vt_tiles.pop((h, si))
                    for ci in range(cn):
                        ct = c0 + ci
                        xt_sb, xi = xt_of_ct[ct]
                        # rt outer on the very first tile: rt0's matmuls only
                        # need the first half-tile DMA
                        first = h == 0 and ct == 0
                        order = (
                            [(rt, bt) for rt in range(nrt) for bt in range(nbt)]
                            if first
                            else [(rt, bt) for bt in range(nbt) for rt in range(nrt)]
                        )
                        for rt, bt in order:
                            lhsT = xt_sb[:, xi, 128 * bt : 128 * (bt + 1)]
                            nc.tensor.matmul(
                                ps_tiles[bt * nrt + rt][:],
                                lhsT,
                                vt_t[:, ci, tn * rt : tn * (rt + 1)],
                                start=(ct == 0),
                                stop=(ct == nct - 1),
                            )
                for bt in range(nbt):
                    for rt in range(nrt):
                        i = bt * nrt + rt
                        st_t = stp.tile([128, tn], bf16, tag="st")
                        # copy (with f32->bf16 convert), split across DVE + ACT
                        if i % 2 == 0:
                            nc.vector.tensor_scalar(
                                st_t[:], ps_tiles[i][:], 0.0, None, op0=Alu.add
                            )
                        else:
                            nc.scalar.copy(st_t[:], ps_tiles[i][:])
                        # h0 stores overlap h1 compute on the 2 HWDGE rings;
                        # h1's are the exposed tail -> spread over 3 rings
                        st_lanes = (
                            (nc.sync, nc.scalar)
                            if h == 0
                            else (nc.sync, nc.scalar, nc.gpsimd)
                        )
                        st_lanes[i % len(st_lanes)].dma_start(
                            out_d[
                                128 * bt : 128 * (bt + 1),
                                rhalf * h + tn * rt : rhalf * h + tn * (rt + 1),
                            ],
                            st_t[:],
                        )
    nc.compile()
    return nc


# ---------------- host-side prep / gather ----------------

def host_prep(x, V, alpha, cfg=CFG_FULL):
    """Build the 8 per-core input maps. Core id = ib*4 + ir."""
    import ml_dtypes

    n, bb, r_sh = cfg["N"], cfg["BB"], cfg["R"]
    x = np.ascontiguousarray(x, dtype=np.float32)
    V = np.ascontiguousarray(V, dtype=np.float32)
    alpha = np.ascontiguousarray(alpha, dtype=np.float32)

    a = dykstra_host(alpha)

    # AT[c, d] = a[d] * V[d, c]; band row c of the sheared gather is the
    # contiguous run AT3[c, n - c : n - c + n + r_sh] (zero-copy strided view)
    AT = np.ascontiguousarray(V.T) * a[None, :]
    AT3 = np.concatenate([AT, AT, AT[:, :r_sh]], axis=1)
    AT3 = np.ascontiguousarray(AT3)
    pitch = AT3.strides[0]
    isz = AT3.itemsize
    Bview = np.lib.stride_tricks.as_strided(
        AT3[:, n:], shape=(n, n + r_sh), strides=(pitch - isz, isz)
    )
    # Bview[c, m] = AT3[c, n - c + m] = a[(m - c) % n] * V[(m - c) % n, c]
    bts = [
        np.ascontiguousarray(Bview[:, r0 : r0 + r_sh].astype(ml_dtypes.bfloat16))
        for r0 in range(0, n, r_sh)
    ]
    del AT, AT3, Bview

    xb = x.astype(ml_dtypes.bfloat16)
    xtfs = [
        np.ascontiguousarray(xb[b0 : b0 + bb].T) for b0 in range(0, x.shape[0], bb)
    ]

    in_maps = []
    for ib in range(x.shape[0] // bb):
        for ir in range(n // r_sh):
            in_maps.append({"xtf_in": xtfs[ib], "bt_in": bts[ir]})
    return in_maps


_nc_cache = None


def kernel(x, V, alpha):
    """Full-input, full-output entry point. Shards over 8 NeuronCores."""
    from concourse import bass_utils

    global _nc_cache
    if _nc_cache is None:
        _nc_cache = build_nc(CFG_FULL)
    nc = _nc_cache

    in_maps = host_prep(x, V, alpha, CFG_FULL)
    res = bass_utils.run_bass_kernel_spmd(nc, in_maps, core_ids=list(range(8)))
    kernel.last_results = res

    bb, r_sh = CFG_FULL["BB"], CFG_FULL["R"]
    out = np.empty((BATCH, N), np.float32)
    for core, rmap in enumerate(res.results):
        ib, ir = divmod(core, N // r_sh)
        out[bb * ib : bb * (ib + 1), r_sh * ir : r_sh * (ir + 1)] = np.asarray(
            rmap["out_d"]
        ).astype(np.float32)
    return out


# revision 35
# speedup vs baseline: 1.0260x; 1.0260x over previous
"""Trainium2 Bass kernel: topk-masked pseudo-diagonal linear layer.

Math:  a = dykstra_topk(alpha);  W[r,c] = a[(r-c)%n] * V[(r-c)%n, c];
       out = x @ W.T,   with n = 8192, x [1024, 8192], V [8192, 8192].

Strategy (8 NeuronCores, SPMD, no collectives):
  - 2D shard: 4-way over out-features r (R=2048/core) x 2-way over batch
    (BB=512/core).  Each core computes out[b0:b0+512, r0:r0+2048].
  - Host does the cheap, layout-bound work: Dykstra projection of alpha
    (0.4 MFLOP) and the band gather B[c, j] = a[d] * V[d, c] with
    d = (r0 + j - c) % n, emitted in bf16.  The device is a pure
    streaming matmul at the bf16 PE rate: out_tile = xT_block^T @
    B_tile, accumulated over 64 c-tiles in 8 PSUM banks with pipelined
    bf16 LDWEIGHTS.
  - DMA: one queue sustains only ~75 GB/s, and the h0 phase needs
    ~218 GB/s (B stream + resident-x fill), so bulk loads round-robin
    over 5 queues: sync/scalar HWDGE + SWDGE q0 (dma_start) + SWDGE
    q1/q2 (direct dma_gather of 2KB/1KB rows via an iota row-index
    table).  x loads are sliced into 128-256 KB pieces so B tiles
    behind them in a queue are never delayed much.
"""

import math
import numpy as np

# ---- problem constants (hardcoded; must match reference.py) ----
N = 8192
BATCH = 1024
KTOP = math.ceil((1.0 - 0.9) * N * N / N)  # 820
LR = 0.05
ITERS = 50

CFG_FULL = dict(N=N, BB=512, R=2048, TN=512, GATHER_LANES=0)


def dykstra_host(alpha):
    """Euclidean projection of alpha/LR onto {p: 0<=p<=1, sum p = K} via
    the same 50 Dykstra iterations as the reference (f64 accumulate)."""
    x0 = alpha.astype(np.float64) / LR
    n = x0.shape[0]
    v = x0.copy()
    p = np.zeros_like(v)
    q = np.zeros_like(v)
    for _ in range(ITERS):
        t = v + p
        y = t + (KTOP - t.sum()) / n
        p = t - y
        yq = y + q
        v = np.clip(yq, 0.0, 1.0)
        q = yq - v
    return v.astype(np.float32)


def build_nc(cfg=CFG_FULL):
    """Build + compile the single-core SPMD Bass program."""
    import concourse.bass as bass
    import concourse.tile as tile
    from concourse import bacc, mybir

    f32 = mybir.dt.float32
    bf16 = mybir.dt.bfloat16
    i16 = mybir.dt.int16
    Alu = mybir.AluOpType

    n, bb, r_sh, tn = cfg["N"], cfg["BB"], cfg["R"], cfg["TN"]
    glanes = cfg["GATHER_LANES"]
    rhalf = r_sh // 2
    nct = n // 128          # c-tiles
    nbt = bb // 128         # b-tiles
    nrt = rhalf // tn       # r-subtiles per half
    assert nbt * nrt <= 8
    # x pieces, all on the dedicated scalar ring: small first pieces so the
    # first matmuls start early, 1 MB steady pieces after
    xpieces = (2, 2, 4, 8, 8, 8, 8, 8, 8, 8)
    assert sum(xpieces) == nct

    nc = bacc.Bacc(
        "TRN2", target_bir_lowering=False, debug=False, enable_asserts=False,
        num_swdge_queues=1 + glanes,
    )
    xtf_in = nc.dram_tensor("xtf_in", [n, bb], bf16, kind="ExternalInput").ap()
    bt_in = nc.dram_tensor("bt_in", [n, r_sh], bf16, kind="ExternalInput").ap()
    out_d = nc.dram_tensor("out_d", [bb, r_sh], bf16, kind="ExternalOutput").ap()

    with tile.TileContext(nc) as tc:
        with (
            tc.tile_pool(name="idx", bufs=1) as idxp,
            tc.tile_pool(name="xt", bufs=1) as xtp,
            tc.tile_pool(name="vt", bufs=10) as vtp,
            tc.tile_pool(name="vt0", bufs=1) as vtp0,
            tc.tile_pool(name="ps", bufs=8, space=bass.MemorySpace.PSUM) as psp,
            tc.tile_pool(name="st", bufs=4) as stp,
        ):
            # row-index table for dma_gather lanes: idx[p, c] = 16*c + p
            # (row g of a gather lands at table [p=g%16, col=g//16]); clamp
            # to n-1 so the unused partitions p>=16 stay in bounds
            idx_tab = None
            if glanes:
                idx_tab = idxp.tile([128, n // 16], i16)
                nc.gpsimd.iota(
                    idx_tab[:], pattern=[[16, n // 16]], base=0,
                    channel_multiplier=1,
                )
                nc.gpsimd.tensor_scalar(
                    idx_tab[:], idx_tab[:], float(n - 1), None, op0=Alu.min
                )

            # 5 bulk-load lanes, each ~75 GB/s
            def load(dst, src_rows_dram, row0, nrows, elem, estep, lane):
                """dst[128, nrows/128, elem] <- rows [row0, row0+nrows) of a
                DRAM view whose rows are `elem` bf16 wide with pitch `estep`."""
                if lane <= 2:
                    eng = (nc.sync, nc.scalar, nc.gpsimd)[lane]
                    eng.dma_start(
                        dst, src_rows_dram[row0 : row0 + nrows, :].rearrange(
                            "(ct p) b -> p ct b", p=128
                        )
                    )
                else:
                    nc.gpsimd.dma_gather(
                        dst,
                        src_rows_dram,
                        idx_tab[:, row0 // 16 : row0 // 16 + nrows // 16],
                        num_idxs=nrows,
                        num_idxs_reg=nrows,
                        elem_size=elem,
                        elem_step=estep,
                        queue_num=lane - 2,
                    )

            # resident xT piece tiles
            xt_tiles = []
            xt_of_ct = {}
            ct0 = 0
            for xc, cpc in enumerate(xpieces):
                xt_sb = xtp.tile([128, cpc, bb], bf16, name=f"xt{xc}")
                xt_tiles.append((xt_sb, ct0, cpc))
                for ci in range(cpc):
                    xt_of_ct[ct0 + ci] = (xt_sb, ci)
                ct0 += cpc

            def load_xt(xc):
                xt_sb, c0, cpc = xt_tiles[xc]
                load(xt_sb[:], xtf_in, 128 * c0, 128 * cpc, bb, bb, 1)

            bt_half = [
                bt_in[:, rhalf * h : rhalf * (h + 1)] for h in range(2)
            ]

            # one B tile per c-tile (2-ct segments measured slower)
            vt_segs = {
                0: [(ct, 1) for ct in range(nct)],
                1: [(ct, 1) for ct in range(nct)],
            }
            vt_tiles = {}

            def load_vt(h, si):
                c0, cn = vt_segs[h][si]
                # h0: vt alternates the sync + gpsimd rings (scalar is
                # saturated by the x fill); h1: scalar is free, use 3 rings
                lane = (0, 2)[si % 2] if h == 0 else (0, 2, 1)[si % 3]
                vt_t = vtp.tile([128, cn, rhalf], bf16, tag="vt")
                vt_tiles[(h, si)] = vt_t
                load(vt_t[:], bt_half[h], 128 * c0, 128 * cn, rhalf, r_sh, lane)

            # ---- prefetch ramp: first B tiles + x fill on its own ring ----
            load_vt(0, 0)
            load_vt(0, 1)
            for xc in range(len(xpieces)):
                load_xt(xc)

            # ---- main loop ----
            for h in range(2):
                ps_tiles = [
                    psp.tile([128, tn], f32, tag="mm", name=f"ps_{h}_{i}")
                    for i in range(nbt * nrt)
                ]
                for si, (c0, cn) in enumerate(vt_segs[h]):
                    if (h, si) not in vt_tiles:
                        load_vt(h, si)
                    vt_t = vt_tiles.pop((h, si))
                    for ci in range(cn):
                        ct = c0 + ci
                        xt_sb, xi = xt_of_ct[ct]
                        for bt in range(nbt):
                            lhsT = xt_sb[:, xi, 128 * bt : 128 * (bt + 1)]
                            for rt in range(nrt):
                                nc.tensor.matmul(
                                    ps_tiles[bt * nrt + rt][:],
                                    lhsT,
                                    vt_t[:, ci, tn * rt : tn * (rt + 1)],
                                    start=(ct == 0),
                                    stop=(ct == nct - 1),
                                )
                for bt in range(nbt):
                    for rt in range(nrt):
                        i = bt * nrt + rt
                        st_t = stp.tile([128, tn], bf16, tag="st")
                        # copy (with f32->bf16 convert), split across DVE + ACT
                        if i % 2 == 0:
                            nc.vector.tensor_scalar(
                                st_t[:], ps_tiles[i][:], 0.0, None, op0=Alu.add
                            )
                        else:
                            nc.scalar.copy(st_t[:], ps_tiles[i][:])
                        # stores on the two hardware-DGE rings only (a store
                        # queued on the gpsimd SWDGE ring stalls final drain)
                        (nc.sync if i % 2 == 0 else nc.scalar).dma_start(
                            out_d[
                                128 * bt : 128 * (bt + 1),
                                rhalf * h + tn * rt : rhalf * h + tn * (rt + 1),
                            ],
                            st_t[:],
                        )
    nc.compile()
    return nc


# ---------------- host-side prep / gather ----------------

def host_prep(x, V, alpha, cfg=CFG_FULL):
    """Build the 8 per-core input maps. Core id = ib*4 + ir."""
    import ml_dtypes

    n, bb, r_sh = cfg["N"], cfg["BB"], cfg["R"]
    x = np.ascontiguousarray(x, dtype=np.float32)
    V = np.ascontiguousarray(V, dtype=np.float32)
    alpha = np.ascontiguousarray(alpha, dtype=np.float32)

    a = dykstra_host(alpha)

    # AT[c, d] = a[d] * V[d, c]; band row c of the sheared gather is the
    # contiguous run AT3[c, n - c : n - c + n + r_sh] (zero-copy strided view)
    AT = np.ascontiguousarray(V.T) * a[None, :]
    AT3 = np.concatenate([AT, AT, AT[:, :r_sh]], axis=1)
    AT3 = np.ascontiguousarray(AT3)
    pitch = AT3.strides[0]
    isz = AT3.itemsize
    Bview = np.lib.stride_tricks.as_strided(
        AT3[:, n:], shape=(n, n + r_sh), strides=(pitch - isz, isz)
    )
    # Bview[c, m] = AT3[c, n - c + m] = a[(m - c) % n] * V[(m - c) % n, c]
    bts = [
        np.ascontiguousarray(Bview[:, r0 : r0 + r_sh].astype(ml_dtypes.bfloat16))
        for r0 in range(0, n, r_sh)
    ]
    del AT, AT3, Bview

    xb = x.astype(ml_dtypes.bfloat16)
    xtfs = [
        np.ascontiguousarray(xb[b0 : b0 + bb].T) for b0 in range(0, x.shape[0], bb)
    ]

    in_maps = []
    for ib in range(x.shape[0] // bb):
        for ir in range(n // r_sh):
            in_maps.append({"xtf_in": xtfs[ib], "bt_in": bts[ir]})
    return in_maps


_nc_cache = None


def kernel(x, V, alpha):
    """Full-input, full-output entry point. Shards over 8 NeuronCores."""
    from concourse import bass_utils

    global _nc_cache
    if _nc_cache is None:
        _nc_cache = build_nc(CFG_FULL)
    nc = _nc_cache

    in_maps = host_prep(x, V, alpha, CFG_FULL)
    res = bass_utils.run_bass_kernel_spmd(nc, in_maps, core_ids=list(range(8)))
    kernel.last_results = res

    bb, r_sh = CFG_FULL["BB"], CFG_FULL["R"]
    out = np.empty((BATCH, N), np.float32)
    for core, rmap in enumerate(res.results):
        ib, ir = divmod(core, N // r_sh)
        out[bb * ib : bb * (ib + 1), r_sh * ir : r_sh * (ir + 1)] = np.asarray(
            rmap["out_d"]
        ).astype(np.float32)
    return out
